# revision 11
# baseline (speedup 1.0000x reference)
"""Bilateral-grid slice kernel for Trainium2 (8 NeuronCores, SPMD data-parallel).

Strategy (per core):
  - shard: view v = core//4 owns grids[v]; quarter q = core%4 owns image rows
    [270q, 270(q+1)) of the 1080-row image -> 518400 pixels per core.
  - pixels live in "block layout" [128 partitions, 4050 free] (pixel = p*4050+j).
  - trilinear interp of the (8,16,16) grid:
      hat weights  hz[8], hy[16], hx[16]  with hat(t) = relu(1-|t|)
      S = hz (x) hy  joint one-hot over the 128 (z,y)-cells  (DVE mul, bf16)
      S^T per 128-pixel tile via regular matmul against identity (cheaper
      than transpose-mode and keeps the PE HAM-warm), PSUM -> SBUF bf16
      V[px, (ch,x)] = S @ G3 on the PE in bf16  (G3 = grid [128, 192])
      V -> SBUF bf16 (ACT copy), w2 = V * hx (DVE bf16 2x), reduce over x
      out = A[:, i*4+j] affine-applied to rgb (GPSIMD tensor ops)
"""

import numpy as np
from contextlib import ExitStack

import concourse.bacc as bacc
import concourse.bass as bass
import concourse.tile as tile
import concourse.mybir as mybir
from concourse import masks
from concourse.bass_utils import run_bass_kernel_spmd

F32 = mybir.dt.float32
BF16 = mybir.dt.bfloat16
ALU = mybir.AluOpType
ACTFN = mybir.ActivationFunctionType

# problem geometry (hardcoded per contest rules)
NVIEW, L, GH, GW = 2, 8, 16, 16
IMG_H, IMG_W = 1080, 1920
NCORES = 8
P = 128

ROWS_PER_CORE = IMG_H // 4                      # 270
PIX_PER_CORE = ROWS_PER_CORE * IMG_W            # 518400
CTOT = PIX_PER_CORE // P                        # 4050
CHUNK = 162                                     # free-cols per chunk
NCHUNK = CTOT // CHUNK                          # 25
JGRP = 6                                        # j's per S-product group
SUBJ = 3                                        # j's per PSUM sub-tile
NGRP = CHUNK // JGRP                            # 27

GRAY_R, GRAY_G, GRAY_B = 0.299, 0.587, 0.114


def _ap(base: bass.AP, offset_add: int, free_dims):
    """Raw AP on the same tensor/partitions as `base` with custom free dims."""
    return bass.AP(base.tensor, base.offset + offset_add, [base.ap[0]] + free_dims)


def build_module(ctot=CTOT, chunk=CHUNK, jgrp=JGRP):
    nchunk = ctot // chunk
    ngrp = chunk // jgrp
    assert ctot % chunk == 0 and chunk % jgrp == 0

    nc = bacc.Bacc("TRN2", target_bir_lowering=False, debug=False,
                   num_devices=NCORES)

    xs = nc.dram_tensor("xs", [P, ctot], F32, kind="ExternalInput").ap()
    ys = nc.dram_tensor("ys", [P, ctot], F32, kind="ExternalInput").ap()
    rr = nc.dram_tensor("rr", [P, ctot], F32, kind="ExternalInput").ap()
    gg = nc.dram_tensor("gg", [P, ctot], F32, kind="ExternalInput").ap()
    bb = nc.dram_tensor("bb", [P, ctot], F32, kind="ExternalInput").ap()
    g3d = nc.dram_tensor("g3", [P, 192], F32, kind="ExternalInput").ap()
    # hat-lattice constants pre-expanded per pixel-column position:
    # cstE[0, j*40 + 0:8]   = z lattice 0..7
    # cstE[0, j*40 + 8:24]  = y lattice 0..15
    # cstE[0, j*40 + 24:40] = x lattice 0..15
    cstE = nc.dram_tensor("cstE", [1, 40 * chunk], F32,
                          kind="ExternalInput").ap()
    out = nc.dram_tensor("out", [P, 3 * ctot], F32, kind="ExternalOutput").ap()

    with tile.TileContext(nc) as tc:
        with ExitStack() as ctx:
            cpool = ctx.enter_context(tc.tile_pool(name="const", bufs=1))
            inp = ctx.enter_context(tc.tile_pool(name="inp", bufs=2))
            hatp = ctx.enter_context(tc.tile_pool(name="hat", bufs=2))
            spool = ctx.enter_context(tc.tile_pool(name="sprod", bufs=3))
            stp = ctx.enter_context(
                tc.tile_pool(name="st_ps", bufs=2, space="PSUM"))
            stsb = ctx.enter_context(tc.tile_pool(name="st_sb", bufs=3))
            vps = ctx.enter_context(
                tc.tile_pool(name="v_ps", bufs=2, space="PSUM"))
            vsbp = ctx.enter_context(tc.tile_pool(name="v_sb", bufs=3))
            w2p = ctx.enter_context(tc.tile_pool(name="w2", bufs=2))
            apool = ctx.enter_context(tc.tile_pool(name="acc", bufs=2))
            opool = ctx.enter_context(tc.tile_pool(name="outb", bufs=2))

            # constants
            g3_f = cpool.tile([P, 192], F32)
            nc.sync.dma_start(g3_f[:], g3d)
            g3_sb = cpool.tile([P, 192], BF16)
            nc.vector.tensor_copy(g3_sb[:], g3_f[:])
            # expanded lattice constants, bf16, one per (j, cell) so the hat
            # STT reads step-1 (keeps the DVE 2x mode eligible)
            cstE_f = cpool.tile([P, 40 * chunk], F32)
            nc.sync.dma_start(cstE_f[:],
                              cstE[0:1, :].to_broadcast((P, 40 * chunk)))
            cstE_sb = cpool.tile([P, 40 * chunk], BF16)
            nc.vector.tensor_copy(cstE_sb[:], cstE_f[:])
            ident = cpool.tile([P, P], F32)
            masks.make_identity(nc, ident[:])
            ident_b = cpool.tile([P, P], BF16)
            nc.vector.tensor_copy(ident_b[:], ident[:])

            for ci in range(nchunk):
                cb = ci * chunk
                xt = inp.tile([P, chunk], F32, tag="xt")
                nc.sync.dma_start(xt[:], xs[:, cb:cb + chunk])
                yt = inp.tile([P, chunk], F32, tag="yt")
                nc.sync.dma_start(yt[:], ys[:, cb:cb + chunk])
                rt = inp.tile([P, chunk], F32, tag="rt")
                nc.sync.dma_start(rt[:], rr[:, cb:cb + chunk])
                gt = inp.tile([P, chunk], F32, tag="gt")
                nc.sync.dma_start(gt[:], gg[:, cb:cb + chunk])
                bt = inp.tile([P, chunk], F32, tag="bt")
                nc.sync.dma_start(bt[:], bb[:, cb:cb + chunk])

                # gray precursor (z = t2 * 0.587*7 folded into the hat STT)
                t1 = inp.tile([P, chunk], F32, tag="t1")
                nc.vector.scalar_tensor_tensor(
                    t1[:], rt[:], GRAY_R / GRAY_G, gt[:],
                    op0=ALU.mult, op1=ALU.add)
                t2 = inp.tile([P, chunk], F32, tag="t2")
                nc.vector.scalar_tensor_tensor(
                    t2[:], bt[:], GRAY_B / GRAY_G, t1[:],
                    op0=ALU.mult, op1=ALU.add)
                t2b = inp.tile([P, chunk], BF16, tag="t2b")
                nc.vector.tensor_copy(t2b[:], t2[:])
                xtb = inp.tile([P, chunk], BF16, tag="xtb")
                nc.vector.tensor_copy(xtb[:], xt[:])
                ytb = inp.tile([P, chunk], BF16, tag="ytb")
                nc.vector.tensor_copy(ytb[:], yt[:])

                # hat argument tiles. hz/hy are built cell-major (cell, j) so
                # every operand's innermost dim is step-1 -> DVE 2x_1P fires;
                # hx stays j-major (j, x) because the w2-mul needs x inner.
                hzc = hatp.tile([P, 8 * chunk], BF16, tag="hzc")
                nc.vector.scalar_tensor_tensor(
                    hzc[:].rearrange("p (z j) -> p z j", z=8),
                    t2b[:].unsqueeze(1).broadcast_to((P, 8, chunk)),
                    GRAY_G * (L - 1),
                    cstE_sb[:, 0:8 * chunk].rearrange(
                        "p (z j) -> p z j", z=8),
                    op0=ALU.mult, op1=ALU.subtract)
                hyc = hatp.tile([P, 16 * chunk], BF16, tag="hyc")
                nc.vector.scalar_tensor_tensor(
                    hyc[:].rearrange("p (y j) -> p y j", y=16),
                    ytb[:].unsqueeze(1).broadcast_to((P, 16, chunk)),
                    float(GH - 1),
                    cstE_sb[:, 8 * chunk:24 * chunk].rearrange(
                        "p (y j) -> p y j", y=16),
                    op0=ALU.mult, op1=ALU.subtract)
                hx = hatp.tile([P, 16 * chunk], BF16, tag="hx")
                nc.vector.scalar_tensor_tensor(
                    hx[:].rearrange("p (j x) -> p j x", x=16),
                    xtb[:].unsqueeze(2).broadcast_to((P, chunk, 16)),
                    float(GW - 1),
                    cstE_sb[:, 24 * chunk:40 * chunk].rearrange(
                        "p (j x) -> p j x", x=16),
                    op0=ALU.mult, op1=ALU.subtract)

                # hat(t) = relu(1 - |t|), in place on ACT
                for h in (hzc, hyc, hx):
                    nc.scalar.activation(h[:], h[:], ACTFN.Abs)
                    nc.scalar.activation(h[:], h[:], ACTFN.Relu,
                                         bias=1.0, scale=-1.0)

                a_ch = apool.tile([P, 12 * chunk], F32, tag="a_ch")

                for g in range(ngrp):
                    jb = g * jgrp
                    # S one-hot product [pix, (j, z, y)] bf16
                    sg = spool.tile([P, jgrp * P], BF16, tag="sg")
                    nc.vector.tensor_tensor(
                        sg[:].rearrange("p (j z y) -> p j z y", j=jgrp, z=8),
                        _ap(hzc[:], jb, [[1, jgrp], [chunk, 8], [0, 16]]),
                        _ap(hyc[:], jb, [[1, jgrp], [0, 8], [chunk, 16]]),
                        op=ALU.mult)

                    for h in range(jgrp // SUBJ):
                        qb = h * SUBJ
                        # S^T via regular matmuls against identity (PSUM out)
                        st_ps = stp.tile([P, SUBJ * P], F32)
                        for q in range(SUBJ):
                            nc.tensor.matmul(
                                _ap(st_ps[:], q * P, [[1, P]]),
                                lhsT=sg[:, (qb + q) * P:(qb + q + 1) * P],
                                rhs=ident_b[:], start=True, stop=True)
                        st_sb = stsb.tile([P, SUBJ * P], BF16)
                        nc.scalar.copy(st_sb[:], st_ps[:])

                        # V[pix, (ch, x)] = S @ G3, bf16 weights (FWL).
                        # 256-elem slots: each matmul's 192-wide output stays
                        # inside one 512-float PSUM bank (256*3=768 -> 2 banks)
                        vt = vps.tile([P, SUBJ * 256], F32)
                        for q in range(SUBJ):
                            nc.tensor.matmul(
                                _ap(vt[:], q * 256, [[1, 192]]),
                                lhsT=_ap(st_sb[:], q * P, [[1, P]]),
                                rhs=g3_sb[:], start=True, stop=True)
                        v_sb = vsbp.tile([P, SUBJ * 192], BF16)
                        nc.scalar.copy(
                            v_sb[:].rearrange("p (q c) -> p q c", q=SUBJ),
                            _ap(vt[:], 0, [[256, SUBJ], [1, 192]]))

                        # w2 = V * hx (bf16; inner dim = 16 contiguous x)
                        w2 = w2p.tile([P, SUBJ * 192], BF16)
                        nc.vector.tensor_tensor(
                            _ap(w2[:], 0, [[192, SUBJ], [16, 12], [1, 16]]),
                            _ap(v_sb[:], 0, [[192, SUBJ], [16, 12], [1, 16]]),
                            _ap(hx[:], (jb + qb) * 16,
                                [[16, SUBJ], [0, 12], [1, 16]]),
                            op=ALU.mult)
                        nc.vector.tensor_reduce(
                            _ap(a_ch[:], (jb + qb) * 12,
                                [[12, SUBJ], [1, 12]]),
                            _ap(w2[:], 0, [[192, SUBJ], [16, 12], [1, 16]]),
                            axis=mybir.AxisListType.X, op=ALU.add)

                # affine apply on GPSIMD:
                # out_i = A[4i]*r + A[4i+1]*g + A[4i+2]*b + A[4i+3]
                ot = opool.tile([P, 3 * chunk], F32, tag="ot")
                rgbt = (rt, gt, bt)
                for i in range(3):
                    m = []
                    for j in range(3):
                        mj = opool.tile([P, chunk], F32, tag=f"m{j}")
                        nc.gpsimd.tensor_tensor(
                            mj[:],
                            _ap(a_ch[:], 4 * i + j, [[12, chunk]]),
                            rgbt[j][:], op=ALU.mult)
                        m.append(mj)
                    s1 = opool.tile([P, chunk], F32, tag="s1")
                    nc.gpsimd.tensor_tensor(s1[:], m[0][:], m[1][:], op=ALU.add)
                    s2 = opool.tile([P, chunk], F32, tag="s2")
                    nc.gpsimd.tensor_tensor(
                        s2[:], m[2][:],
                        _ap(a_ch[:], 4 * i + 3, [[12, chunk]]), op=ALU.add)
                    nc.gpsimd.tensor_tensor(
                        _ap(ot[:], i, [[3, chunk]]), s1[:], s2[:], op=ALU.add)

                nc.sync.dma_start(out[:, 3 * cb:3 * (cb + chunk)], ot[:])

    nc.compile()
    return nc


_NC_CACHE = {}


def _get_module():
    key = (CTOT, CHUNK, JGRP)
    if key not in _NC_CACHE:
        _NC_CACHE[key] = build_module()
    return _NC_CACHE[key]


def _make_core_inputs(grids, coords, rgb, ctot=CTOT, chunk=CHUNK):
    """Per-core input dicts (numpy layout prep only)."""
    # cell-major lattices for z/y (cell varies slow, j fast); j-major for x
    z_cm = np.repeat(np.arange(8, dtype=np.float32), chunk)       # (8*chunk,)
    y_cm = np.repeat(np.arange(16, dtype=np.float32), chunk)      # (16*chunk,)
    x_jm = np.tile(np.arange(16, dtype=np.float32), chunk)        # (chunk*16,)
    cstE = np.concatenate([z_cm, y_cm, x_jm]).reshape(1, 40 * chunk)
    cstE = cstE.astype(np.float32)
    in_maps = []
    for core in range(NCORES):
        v, q = divmod(core, 4)
        r0, r1 = ROWS_PER_CORE * q, ROWS_PER_CORE * (q + 1)
        blk = lambda a: np.ascontiguousarray(a.reshape(P, ctot), np.float32)
        # G3[(zc*16+yc), (ch*16 + xc)] = grids[v, ch, zc, yc, xc]
        g3 = np.ascontiguousarray(
            grids[v].transpose(1, 2, 0, 3).reshape(P, 192), np.float32)
        in_maps.append({
            "xs": blk(coords[v, 0, r0:r1, :, 0]),
            "ys": blk(coords[v, 0, r0:r1, :, 1]),
            "rr": blk(rgb[v, 0, r0:r1, :, 0]),
            "gg": blk(rgb[v, 0, r0:r1, :, 1]),
            "bb": blk(rgb[v, 0, r0:r1, :, 2]),
            "g3": g3,
            "cstE": cstE,
        })
    return in_maps


def _run(grids, coords, rgb, trace=False):
    nc = _get_module()
    in_maps = _make_core_inputs(grids, coords, rgb)
    res = run_bass_kernel_spmd(nc, in_maps, core_ids=list(range(NCORES)),
                               trace=trace)
    outs = []
    for core in range(NCORES):
        o = res.results[core]["out"]
        outs.append(o.reshape(P, CTOT, 3).reshape(ROWS_PER_CORE, IMG_W, 3))
    full = np.empty((NVIEW, 1, IMG_H, IMG_W, 3), np.float32)
    for core in range(NCORES):
        v, q = divmod(core, 4)
        full[v, 0, ROWS_PER_CORE * q:ROWS_PER_CORE * (q + 1)] = outs[core]
    return full, res


def kernel(grids, coords, rgb):
    full, _ = _run(np.asarray(grids), np.asarray(coords), np.asarray(rgb))
    return full


# revision 18
# speedup vs baseline: 1.1475x; 1.1475x over previous
"""Bilateral-grid slice kernel for Trainium2 (8 NeuronCores, SPMD data-parallel).

Strategy (per core):
  - shard: view v = core//4 owns grids[v]; quarter q = core%4 owns image rows
    [270q, 270(q+1)) of the 1080-row image -> 518400 pixels per core.
  - pixels live in "block layout" [128 partitions, 4050 free] (pixel = p*4050+j).
  - trilinear interp of the (8,16,16) grid:
      hat weights  hz[8], hy[16], hx[16]  with hat(t) = relu(1-|t|)
      S = hz (x) hy  joint one-hot over the 128 (z,y)-cells  (DVE mul, bf16)
      S^T per 128-pixel tile via regular matmul against identity (cheaper
      than transpose-mode and keeps the PE HAM-warm), PSUM -> SBUF bf16
      V[px, (ch,x)] = S @ G3 on the PE in bf16  (G3 = grid [128, 192])
      V -> SBUF bf16 (ACT copy), w2 = V * hx (DVE bf16 2x), reduce over x
      out = A[:, i*4+j] affine-applied to rgb (GPSIMD tensor ops)
"""

import numpy as np
from contextlib import ExitStack

import concourse.bacc as bacc
import concourse.bass as bass
import concourse.tile as tile
import concourse.mybir as mybir
from concourse import masks
from concourse.bass_utils import run_bass_kernel_spmd

F32 = mybir.dt.float32
BF16 = mybir.dt.bfloat16
ALU = mybir.AluOpType
ACTFN = mybir.ActivationFunctionType

# problem geometry (hardcoded per contest rules)
NVIEW, L, GH, GW = 2, 8, 16, 16
IMG_H, IMG_W = 1080, 1920
NCORES = 8
P = 128

ROWS_PER_CORE = IMG_H // 4                      # 270
PIX_PER_CORE = ROWS_PER_CORE * IMG_W            # 518400
CTOT = PIX_PER_CORE // P                        # 4050
CHUNK = 162                                     # free-cols per chunk
NCHUNK = CTOT // CHUNK                          # 25
JGRP = 6                                        # j's per S-product group
SUBJ = 3                                        # j's per PSUM sub-tile
MACJ = 18                                       # j's per mul/reduce macro tile
NGRP = CHUNK // JGRP                            # 27

GRAY_R, GRAY_G, GRAY_B = 0.299, 0.587, 0.114


def _ap(base: bass.AP, offset_add: int, free_dims):
    """Raw AP on the same tensor/partitions as `base` with custom free dims."""
    return bass.AP(base.tensor, base.offset + offset_add, [base.ap[0]] + free_dims)


def build_module(ctot=CTOT, chunk=CHUNK, jgrp=JGRP):
    nchunk = ctot // chunk
    ngrp = chunk // jgrp
    assert ctot % chunk == 0 and chunk % jgrp == 0

    nc = bacc.Bacc("TRN2", target_bir_lowering=False, debug=False,
                   num_devices=NCORES)

    xs = nc.dram_tensor("xs", [P, ctot], F32, kind="ExternalInput").ap()
    ys = nc.dram_tensor("ys", [P, ctot], F32, kind="ExternalInput").ap()
    rr = nc.dram_tensor("rr", [P, ctot], F32, kind="ExternalInput").ap()
    gg = nc.dram_tensor("gg", [P, ctot], F32, kind="ExternalInput").ap()
    bb = nc.dram_tensor("bb", [P, ctot], F32, kind="ExternalInput").ap()
    g3d = nc.dram_tensor("g3", [P, 192], F32, kind="ExternalInput").ap()
    # hat-lattice constants pre-expanded per pixel-column position:
    # cstE[0, j*40 + 0:8]   = z lattice 0..7
    # cstE[0, j*40 + 8:24]  = y lattice 0..15
    # cstE[0, j*40 + 24:40] = x lattice 0..15
    cstE = nc.dram_tensor("cstE", [1, 40 * chunk], F32,
                          kind="ExternalInput").ap()
    out = nc.dram_tensor("out", [P, 3 * ctot], F32, kind="ExternalOutput").ap()

    with tile.TileContext(nc) as tc:
        with ExitStack() as ctx:
            cpool = ctx.enter_context(tc.tile_pool(name="const", bufs=1))
            inp = ctx.enter_context(tc.tile_pool(name="inp", bufs=2))
            hatp = ctx.enter_context(tc.tile_pool(name="hat", bufs=2))
            spool = ctx.enter_context(tc.tile_pool(name="sprod", bufs=3))
            stp = ctx.enter_context(
                tc.tile_pool(name="st_ps", bufs=2, space="PSUM"))
            stsb = ctx.enter_context(tc.tile_pool(name="st_sb", bufs=3))
            vps = ctx.enter_context(
                tc.tile_pool(name="v_ps", bufs=2, space="PSUM"))
            vsbp = ctx.enter_context(tc.tile_pool(name="v_sb", bufs=3))
            w2p = ctx.enter_context(tc.tile_pool(name="w2", bufs=2))
            apool = ctx.enter_context(tc.tile_pool(name="acc", bufs=2))
            opool = ctx.enter_context(tc.tile_pool(name="outb", bufs=2))

            # constants
            g3_f = cpool.tile([P, 192], F32)
            nc.sync.dma_start(g3_f[:], g3d)
            g3_sb = cpool.tile([P, 192], BF16)
            nc.vector.tensor_copy(g3_sb[:], g3_f[:])
            # expanded lattice constants, bf16, one per (j, cell) so the hat
            # STT reads step-1 (keeps the DVE 2x mode eligible)
            cstE_f = cpool.tile([P, 40 * chunk], F32)
            nc.sync.dma_start(cstE_f[:],
                              cstE[0:1, :].to_broadcast((P, 40 * chunk)))
            cstE_sb = cpool.tile([P, 40 * chunk], BF16)
            nc.vector.tensor_copy(cstE_sb[:], cstE_f[:])
            ident = cpool.tile([P, P], F32)
            masks.make_identity(nc, ident[:])
            ident_b = cpool.tile([P, P], BF16)
            nc.vector.tensor_copy(ident_b[:], ident[:])

            for ci in range(nchunk):
                cb = ci * chunk
                xt = inp.tile([P, chunk], F32, tag="xt")
                nc.sync.dma_start(xt[:], xs[:, cb:cb + chunk])
                yt = inp.tile([P, chunk], F32, tag="yt")
                nc.sync.dma_start(yt[:], ys[:, cb:cb + chunk])
                rt = inp.tile([P, chunk], F32, tag="rt")
                nc.sync.dma_start(rt[:], rr[:, cb:cb + chunk])
                gt = inp.tile([P, chunk], F32, tag="gt")
                nc.sync.dma_start(gt[:], gg[:, cb:cb + chunk])
                bt = inp.tile([P, chunk], F32, tag="bt")
                nc.sync.dma_start(bt[:], bb[:, cb:cb + chunk])

                # gray precursor (z = t2 * 0.587*7 folded into the hat STT)
                t1 = inp.tile([P, chunk], F32, tag="t1")
                nc.vector.scalar_tensor_tensor(
                    t1[:], rt[:], GRAY_R / GRAY_G, gt[:],
                    op0=ALU.mult, op1=ALU.add)
                t2 = inp.tile([P, chunk], F32, tag="t2")
                nc.vector.scalar_tensor_tensor(
                    t2[:], bt[:], GRAY_B / GRAY_G, t1[:],
                    op0=ALU.mult, op1=ALU.add)
                t2b = inp.tile([P, chunk], BF16, tag="t2b")
                nc.vector.tensor_copy(t2b[:], t2[:])
                xtb = inp.tile([P, chunk], BF16, tag="xtb")
                nc.vector.tensor_copy(xtb[:], xt[:])
                ytb = inp.tile([P, chunk], BF16, tag="ytb")
                nc.vector.tensor_copy(ytb[:], yt[:])

                # hat argument tiles, free layout (j, cell) j-major, bf16
                hz = hatp.tile([P, 8 * chunk], BF16, tag="hz")
                nc.vector.scalar_tensor_tensor(
                    hz[:].rearrange("p (j z) -> p j z", z=8),
                    t2b[:].unsqueeze(2).broadcast_to((P, chunk, 8)),
                    GRAY_G * (L - 1),
                    cstE_sb[:, 0:8 * chunk].rearrange(
                        "p (j z) -> p j z", z=8),
                    op0=ALU.mult, op1=ALU.subtract)
                hy = hatp.tile([P, 16 * chunk], BF16, tag="hy")
                nc.vector.scalar_tensor_tensor(
                    hy[:].rearrange("p (j y) -> p j y", y=16),
                    ytb[:].unsqueeze(2).broadcast_to((P, chunk, 16)),
                    float(GH - 1),
                    cstE_sb[:, 8 * chunk:24 * chunk].rearrange(
                        "p (j y) -> p j y", y=16),
                    op0=ALU.mult, op1=ALU.subtract)
                hx = hatp.tile([P, 16 * chunk], BF16, tag="hx")
                nc.vector.scalar_tensor_tensor(
                    hx[:].rearrange("p (j x) -> p j x", x=16),
                    xtb[:].unsqueeze(2).broadcast_to((P, chunk, 16)),
                    float(GW - 1),
                    cstE_sb[:, 24 * chunk:40 * chunk].rearrange(
                        "p (j x) -> p j x", x=16),
                    op0=ALU.mult, op1=ALU.subtract)

                # hat(t) = relu(1 - |t|), in place on ACT
                for h in (hz, hy, hx):
                    nc.scalar.activation(h[:], h[:], ACTFN.Abs)
                    nc.scalar.activation(h[:], h[:], ACTFN.Relu,
                                         bias=1.0, scale=-1.0)

                a_ch = apool.tile([P, 12 * chunk], F32, tag="a_ch")

                for mg in range(chunk // MACJ):
                    mb = mg * MACJ
                    # V for MACJ j's accumulates here (bf16), then one big
                    # mul + reduce amortizes the per-instruction overhead
                    v_sb = vsbp.tile([P, MACJ * 192], BF16)

                    for g in range(MACJ // jgrp):
                        jb = mb + g * jgrp
                        # S one-hot product [pix, (j, z, y)] bf16; every 3rd
                        # group runs on the (otherwise idle) GPSIMD engine
                        sg = spool.tile([P, jgrp * P], BF16, tag="sg")
                        s_eng = nc.gpsimd if g % 3 == 2 else nc.vector
                        s_eng.tensor_tensor(
                            sg[:].rearrange("p (j z y) -> p j z y",
                                            j=jgrp, z=8),
                            _ap(hz[:], jb * 8, [[8, jgrp], [1, 8], [0, 16]]),
                            _ap(hy[:], jb * 16,
                                [[16, jgrp], [0, 8], [1, 16]]),
                            op=ALU.mult)

                        # V PSUM tile for the whole 6j group (3 banks);
                        # 256-elem slots keep each 192-wide matmul output
                        # inside one 512-float PSUM bank
                        vt = vps.tile([P, jgrp * 256], F32)
                        for h in range(jgrp // SUBJ):
                            qb = h * SUBJ
                            # S^T via matmuls against identity (PSUM out)
                            st_ps = stp.tile([P, SUBJ * P], F32)
                            for q in range(SUBJ):
                                nc.tensor.matmul(
                                    _ap(st_ps[:], q * P, [[1, P]]),
                                    lhsT=sg[:, (qb + q) * P:(qb + q + 1) * P],
                                    rhs=ident_b[:], start=True, stop=True)
                            st_sb = stsb.tile([P, SUBJ * P], BF16)
                            nc.scalar.copy(st_sb[:], st_ps[:])

                            # V[pix, (ch, x)] = S @ G3, bf16 weights (FWL)
                            for q in range(SUBJ):
                                nc.tensor.matmul(
                                    _ap(vt[:], (qb + q) * 256, [[1, 192]]),
                                    lhsT=_ap(st_sb[:], q * P, [[1, P]]),
                                    rhs=g3_sb[:], start=True, stop=True)
                        nc.scalar.copy(
                            _ap(v_sb[:], g * jgrp * 192,
                                [[192, jgrp], [1, 192]]),
                            _ap(vt[:], 0, [[256, jgrp], [1, 192]]))

                    # w2 = V * hx (bf16 2x; inner dim = 16 contiguous x)
                    w2 = w2p.tile([P, MACJ * 192], BF16)
                    nc.vector.tensor_tensor(
                        _ap(w2[:], 0, [[192, MACJ], [16, 12], [1, 16]]),
                        _ap(v_sb[:], 0, [[192, MACJ], [16, 12], [1, 16]]),
                        _ap(hx[:], mb * 16, [[16, MACJ], [0, 12], [1, 16]]),
                        op=ALU.mult)
                    nc.vector.tensor_reduce(
                        _ap(a_ch[:], mb * 12, [[12, MACJ], [1, 12]]),
                        _ap(w2[:], 0, [[192, MACJ], [16, 12], [1, 16]]),
                        axis=mybir.AxisListType.X, op=ALU.add)

                # affine apply on GPSIMD:
                # out_i = A[4i]*r + A[4i+1]*g + A[4i+2]*b + A[4i+3]
                ot = opool.tile([P, 3 * chunk], F32, tag="ot")
                rgbt = (rt, gt, bt)
                for i in range(3):
                    m = []
                    for j in range(3):
                        mj = opool.tile([P, chunk], F32, tag=f"m{j}")
                        nc.gpsimd.tensor_tensor(
                            mj[:],
                            _ap(a_ch[:], 4 * i + j, [[12, chunk]]),
                            rgbt[j][:], op=ALU.mult)
                        m.append(mj)
                    s1 = opool.tile([P, chunk], F32, tag="s1")
                    nc.gpsimd.tensor_tensor(s1[:], m[0][:], m[1][:], op=ALU.add)
                    s2 = opool.tile([P, chunk], F32, tag="s2")
                    nc.gpsimd.tensor_tensor(
                        s2[:], m[2][:],
                        _ap(a_ch[:], 4 * i + 3, [[12, chunk]]), op=ALU.add)
                    nc.gpsimd.tensor_tensor(
                        _ap(ot[:], i, [[3, chunk]]), s1[:], s2[:], op=ALU.add)

                nc.sync.dma_start(out[:, 3 * cb:3 * (cb + chunk)], ot[:])

    nc.compile()
    return nc


_NC_CACHE = {}


def _get_module():
    key = (CTOT, CHUNK, JGRP)
    if key not in _NC_CACHE:
        _NC_CACHE[key] = build_module()
    return _NC_CACHE[key]


def _make_core_inputs(grids, coords, rgb, ctot=CTOT, chunk=CHUNK):
    """Per-core input dicts (numpy layout prep only)."""
    # j-major lattices, one copy per pixel column (keeps STT reads step-1)
    z_jm = np.tile(np.arange(8, dtype=np.float32), chunk)
    y_jm = np.tile(np.arange(16, dtype=np.float32), chunk)
    x_jm = np.tile(np.arange(16, dtype=np.float32), chunk)
    cstE = np.concatenate([z_jm, y_jm, x_jm]).reshape(1, 40 * chunk)
    cstE = cstE.astype(np.float32)
    in_maps = []
    for core in range(NCORES):
        v, q = divmod(core, 4)
        r0, r1 = ROWS_PER_CORE * q, ROWS_PER_CORE * (q + 1)
        blk = lambda a: np.ascontiguousarray(a.reshape(P, ctot), np.float32)
        # G3[(zc*16+yc), (ch*16 + xc)] = grids[v, ch, zc, yc, xc]
        g3 = np.ascontiguousarray(
            grids[v].transpose(1, 2, 0, 3).reshape(P, 192), np.float32)
        in_maps.append({
            "xs": blk(coords[v, 0, r0:r1, :, 0]),
            "ys": blk(coords[v, 0, r0:r1, :, 1]),
            "rr": blk(rgb[v, 0, r0:r1, :, 0]),
            "gg": blk(rgb[v, 0, r0:r1, :, 1]),
            "bb": blk(rgb[v, 0, r0:r1, :, 2]),
            "g3": g3,
            "cstE": cstE,
        })
    return in_maps


def _run(grids, coords, rgb, trace=False):
    nc = _get_module()
    in_maps = _make_core_inputs(grids, coords, rgb)
    res = run_bass_kernel_spmd(nc, in_maps, core_ids=list(range(NCORES)),
                               trace=trace)
    outs = []
    for core in range(NCORES):
        o = res.results[core]["out"]
        outs.append(o.reshape(P, CTOT, 3).reshape(ROWS_PER_CORE, IMG_W, 3))
    full = np.empty((NVIEW, 1, IMG_H, IMG_W, 3), np.float32)
    for core in range(NCORES):
        v, q = divmod(core, 4)
        full[v, 0, ROWS_PER_CORE * q:ROWS_PER_CORE * (q + 1)] = outs[core]
    return full, res


def kernel(grids, coords, rgb):
    full, _ = _run(np.asarray(grids), np.asarray(coords), np.asarray(rgb))
    return full


# revision 19
# speedup vs baseline: 1.1675x; 1.0174x over previous
"""Bilateral-grid slice kernel for Trainium2 (8 NeuronCores, SPMD data-parallel).

Strategy (per core):
  - shard: view v = core//4 owns grids[v]; quarter q = core%4 owns image rows
    [270q, 270(q+1)) of the 1080-row image -> 518400 pixels per core.
  - pixels live in "block layout" [128 partitions, 4050 free] (pixel = p*4050+j).
  - trilinear interp of the (8,16,16) grid:
      hat weights  hz[8], hy[16], hx[16]  with hat(t) = relu(1-|t|)
      S = hz (x) hy  joint one-hot over the 128 (z,y)-cells  (DVE mul, bf16)
      S^T per 128-pixel tile via regular matmul against identity (cheaper
      than transpose-mode and keeps the PE HAM-warm), PSUM -> SBUF bf16
      V[px, (ch,x)] = S @ G3 on the PE in bf16  (G3 = grid [128, 192])
      V -> SBUF bf16 (ACT copy), w2 = V * hx (DVE bf16 2x), reduce over x
      out = A[:, i*4+j] affine-applied to rgb (GPSIMD tensor ops)
"""

import numpy as np
from contextlib import ExitStack

import concourse.bacc as bacc
import concourse.bass as bass
import concourse.tile as tile
import concourse.mybir as mybir
from concourse import masks
from concourse.bass_utils import run_bass_kernel_spmd

F32 = mybir.dt.float32
BF16 = mybir.dt.bfloat16
ALU = mybir.AluOpType
ACTFN = mybir.ActivationFunctionType

# problem geometry (hardcoded per contest rules)
NVIEW, L, GH, GW = 2, 8, 16, 16
IMG_H, IMG_W = 1080, 1920
NCORES = 8
P = 128

ROWS_PER_CORE = IMG_H // 4                      # 270
PIX_PER_CORE = ROWS_PER_CORE * IMG_W            # 518400
CTOT = PIX_PER_CORE // P                        # 4050
CHUNK = 162                                     # free-cols per chunk
NCHUNK = CTOT // CHUNK                          # 25
JGRP = 6                                        # j's per S-product group
SUBJ = 3                                        # j's per PSUM sub-tile
MACJ = 18                                       # j's per mul/reduce macro tile
NGRP = CHUNK // JGRP                            # 27

GRAY_R, GRAY_G, GRAY_B = 0.299, 0.587, 0.114


def _ap(base: bass.AP, offset_add: int, free_dims):
    """Raw AP on the same tensor/partitions as `base` with custom free dims."""
    return bass.AP(base.tensor, base.offset + offset_add, [base.ap[0]] + free_dims)


def build_module(ctot=CTOT, chunk=CHUNK, jgrp=JGRP):
    nchunk = ctot // chunk
    ngrp = chunk // jgrp
    assert ctot % chunk == 0 and chunk % jgrp == 0

    nc = bacc.Bacc("TRN2", target_bir_lowering=False, debug=False,
                   num_devices=NCORES)

    xs = nc.dram_tensor("xs", [P, ctot], F32, kind="ExternalInput").ap()
    ys = nc.dram_tensor("ys", [P, ctot], F32, kind="ExternalInput").ap()
    rr = nc.dram_tensor("rr", [P, ctot], F32, kind="ExternalInput").ap()
    gg = nc.dram_tensor("gg", [P, ctot], F32, kind="ExternalInput").ap()
    bb = nc.dram_tensor("bb", [P, ctot], F32, kind="ExternalInput").ap()
    g3d = nc.dram_tensor("g3", [P, 192], F32, kind="ExternalInput").ap()
    # hat-lattice constants pre-expanded per pixel-column position:
    # cstE[0, j*40 + 0:8]   = z lattice 0..7
    # cstE[0, j*40 + 8:24]  = y lattice 0..15
    # cstE[0, j*40 + 24:40] = x lattice 0..15
    cstE = nc.dram_tensor("cstE", [1, 40 * chunk], F32,
                          kind="ExternalInput").ap()
    out = nc.dram_tensor("out", [P, 3 * ctot], F32, kind="ExternalOutput").ap()

    with tile.TileContext(nc) as tc:
        with ExitStack() as ctx:
            cpool = ctx.enter_context(tc.tile_pool(name="const", bufs=1))
            inp = ctx.enter_context(tc.tile_pool(name="inp", bufs=2))
            hatp = ctx.enter_context(tc.tile_pool(name="hat", bufs=2))
            spool = ctx.enter_context(tc.tile_pool(name="sprod", bufs=3))
            stp = ctx.enter_context(
                tc.tile_pool(name="st_ps", bufs=2, space="PSUM"))
            stsb = ctx.enter_context(tc.tile_pool(name="st_sb", bufs=3))
            vps = ctx.enter_context(
                tc.tile_pool(name="v_ps", bufs=2, space="PSUM"))
            vsbp = ctx.enter_context(tc.tile_pool(name="v_sb", bufs=3))
            w2p = ctx.enter_context(tc.tile_pool(name="w2", bufs=2))
            apool = ctx.enter_context(tc.tile_pool(name="acc", bufs=2))
            opool = ctx.enter_context(tc.tile_pool(name="outb", bufs=2))

            # constants
            g3_f = cpool.tile([P, 192], F32)
            nc.sync.dma_start(g3_f[:], g3d)
            g3_sb = cpool.tile([P, 192], BF16)
            nc.vector.tensor_copy(g3_sb[:], g3_f[:])
            # expanded lattice constants, bf16, one per (j, cell) so the hat
            # STT reads step-1 (keeps the DVE 2x mode eligible)
            cstE_f = cpool.tile([P, 40 * chunk], F32)
            nc.sync.dma_start(cstE_f[:],
                              cstE[0:1, :].to_broadcast((P, 40 * chunk)))
            cstE_sb = cpool.tile([P, 40 * chunk], BF16)
            nc.vector.tensor_copy(cstE_sb[:], cstE_f[:])
            ident = cpool.tile([P, P], F32)
            masks.make_identity(nc, ident[:])
            ident_b = cpool.tile([P, P], BF16)
            nc.vector.tensor_copy(ident_b[:], ident[:])

            for ci in range(nchunk):
                cb = ci * chunk
                xt = inp.tile([P, chunk], F32, tag="xt")
                nc.sync.dma_start(xt[:], xs[:, cb:cb + chunk])
                yt = inp.tile([P, chunk], F32, tag="yt")
                nc.sync.dma_start(yt[:], ys[:, cb:cb + chunk])
                rt = inp.tile([P, chunk], F32, tag="rt")
                nc.sync.dma_start(rt[:], rr[:, cb:cb + chunk])
                gt = inp.tile([P, chunk], F32, tag="gt")
                nc.sync.dma_start(gt[:], gg[:, cb:cb + chunk])
                bt = inp.tile([P, chunk], F32, tag="bt")
                nc.sync.dma_start(bt[:], bb[:, cb:cb + chunk])

                # gray precursor (z = t2 * 0.587*7 folded into the hat STT)
                t1 = inp.tile([P, chunk], F32, tag="t1")
                nc.vector.scalar_tensor_tensor(
                    t1[:], rt[:], GRAY_R / GRAY_G, gt[:],
                    op0=ALU.mult, op1=ALU.add)
                t2 = inp.tile([P, chunk], F32, tag="t2")
                nc.vector.scalar_tensor_tensor(
                    t2[:], bt[:], GRAY_B / GRAY_G, t1[:],
                    op0=ALU.mult, op1=ALU.add)
                t2b = inp.tile([P, chunk], BF16, tag="t2b")
                nc.vector.tensor_copy(t2b[:], t2[:])
                xtb = inp.tile([P, chunk], BF16, tag="xtb")
                nc.vector.tensor_copy(xtb[:], xt[:])
                ytb = inp.tile([P, chunk], BF16, tag="ytb")
                nc.vector.tensor_copy(ytb[:], yt[:])

                # hat argument tiles, free layout (j, cell) j-major, bf16
                hz = hatp.tile([P, 8 * chunk], BF16, tag="hz")
                nc.vector.scalar_tensor_tensor(
                    hz[:].rearrange("p (j z) -> p j z", z=8),
                    t2b[:].unsqueeze(2).broadcast_to((P, chunk, 8)),
                    GRAY_G * (L - 1),
                    cstE_sb[:, 0:8 * chunk].rearrange(
                        "p (j z) -> p j z", z=8),
                    op0=ALU.mult, op1=ALU.subtract)
                hy = hatp.tile([P, 16 * chunk], BF16, tag="hy")
                nc.vector.scalar_tensor_tensor(
                    hy[:].rearrange("p (j y) -> p j y", y=16),
                    ytb[:].unsqueeze(2).broadcast_to((P, chunk, 16)),
                    float(GH - 1),
                    cstE_sb[:, 8 * chunk:24 * chunk].rearrange(
                        "p (j y) -> p j y", y=16),
                    op0=ALU.mult, op1=ALU.subtract)
                hx = hatp.tile([P, 16 * chunk], BF16, tag="hx")
                nc.vector.scalar_tensor_tensor(
                    hx[:].rearrange("p (j x) -> p j x", x=16),
                    xtb[:].unsqueeze(2).broadcast_to((P, chunk, 16)),
                    float(GW - 1),
                    cstE_sb[:, 24 * chunk:40 * chunk].rearrange(
                        "p (j x) -> p j x", x=16),
                    op0=ALU.mult, op1=ALU.subtract)

                # hat(t) = relu(1 - |t|), in place on ACT
                for h in (hz, hy, hx):
                    nc.scalar.activation(h[:], h[:], ACTFN.Abs)
                    nc.scalar.activation(h[:], h[:], ACTFN.Relu,
                                         bias=1.0, scale=-1.0)

                a_ch = apool.tile([P, 12 * chunk], F32, tag="a_ch")

                for mg in range(chunk // MACJ):
                    mb = mg * MACJ
                    # V for MACJ j's accumulates here (bf16), then one big
                    # mul + reduce amortizes the per-instruction overhead
                    v_sb = vsbp.tile([P, MACJ * 192], BF16)

                    for g in range(MACJ // jgrp):
                        jb = mb + g * jgrp
                        # S one-hot product [pix, (j, z, y)] bf16
                        sg = spool.tile([P, jgrp * P], BF16, tag="sg")
                        nc.vector.tensor_tensor(
                            sg[:].rearrange("p (j z y) -> p j z y",
                                            j=jgrp, z=8),
                            _ap(hz[:], jb * 8, [[8, jgrp], [1, 8], [0, 16]]),
                            _ap(hy[:], jb * 16,
                                [[16, jgrp], [0, 8], [1, 16]]),
                            op=ALU.mult)

                        # V PSUM tile for the whole 6j group (3 banks);
                        # 256-elem slots keep each 192-wide matmul output
                        # inside one 512-float PSUM bank
                        vt = vps.tile([P, jgrp * 256], F32)
                        for h in range(jgrp // SUBJ):
                            qb = h * SUBJ
                            # S^T via matmuls against identity (PSUM out)
                            st_ps = stp.tile([P, SUBJ * P], F32)
                            for q in range(SUBJ):
                                nc.tensor.matmul(
                                    _ap(st_ps[:], q * P, [[1, P]]),
                                    lhsT=sg[:, (qb + q) * P:(qb + q + 1) * P],
                                    rhs=ident_b[:], start=True, stop=True)
                            st_sb = stsb.tile([P, SUBJ * P], BF16)
                            nc.scalar.copy(st_sb[:], st_ps[:])

                            # V[pix, (ch, x)] = S @ G3, bf16 weights (FWL)
                            for q in range(SUBJ):
                                nc.tensor.matmul(
                                    _ap(vt[:], (qb + q) * 256, [[1, 192]]),
                                    lhsT=_ap(st_sb[:], q * P, [[1, P]]),
                                    rhs=g3_sb[:], start=True, stop=True)
                        nc.scalar.copy(
                            _ap(v_sb[:], g * jgrp * 192,
                                [[192, jgrp], [1, 192]]),
                            _ap(vt[:], 0, [[256, jgrp], [1, 192]]))

                    # w2 = V * hx (bf16 2x; inner dim = 16 contiguous x)
                    w2 = w2p.tile([P, MACJ * 192], BF16)
                    nc.vector.tensor_tensor(
                        _ap(w2[:], 0, [[192, MACJ], [16, 12], [1, 16]]),
                        _ap(v_sb[:], 0, [[192, MACJ], [16, 12], [1, 16]]),
                        _ap(hx[:], mb * 16, [[16, MACJ], [0, 12], [1, 16]]),
                        op=ALU.mult)
                    nc.vector.tensor_reduce(
                        _ap(a_ch[:], mb * 12, [[12, MACJ], [1, 12]]),
                        _ap(w2[:], 0, [[192, MACJ], [16, 12], [1, 16]]),
                        axis=mybir.AxisListType.X, op=ALU.add)

                # affine apply on GPSIMD:
                # out_i = A[4i]*r + A[4i+1]*g + A[4i+2]*b + A[4i+3]
                ot = opool.tile([P, 3 * chunk], F32, tag="ot")
                rgbt = (rt, gt, bt)
                for i in range(3):
                    m = []
                    for j in range(3):
                        mj = opool.tile([P, chunk], F32, tag=f"m{j}")
                        nc.gpsimd.tensor_tensor(
                            mj[:],
                            _ap(a_ch[:], 4 * i + j, [[12, chunk]]),
                            rgbt[j][:], op=ALU.mult)
                        m.append(mj)
                    s1 = opool.tile([P, chunk], F32, tag="s1")
                    nc.gpsimd.tensor_tensor(s1[:], m[0][:], m[1][:], op=ALU.add)
                    s2 = opool.tile([P, chunk], F32, tag="s2")
                    nc.gpsimd.tensor_tensor(
                        s2[:], m[2][:],
                        _ap(a_ch[:], 4 * i + 3, [[12, chunk]]), op=ALU.add)
                    nc.gpsimd.tensor_tensor(
                        _ap(ot[:], i, [[3, chunk]]), s1[:], s2[:], op=ALU.add)

                nc.sync.dma_start(out[:, 3 * cb:3 * (cb + chunk)], ot[:])

    nc.compile()
    return nc


_NC_CACHE = {}


def _get_module():
    key = (CTOT, CHUNK, JGRP)
    if key not in _NC_CACHE:
        _NC_CACHE[key] = build_module()
    return _NC_CACHE[key]


def _make_core_inputs(grids, coords, rgb, ctot=CTOT, chunk=CHUNK):
    """Per-core input dicts (numpy layout prep only)."""
    # j-major lattices, one copy per pixel column (keeps STT reads step-1)
    z_jm = np.tile(np.arange(8, dtype=np.float32), chunk)
    y_jm = np.tile(np.arange(16, dtype=np.float32), chunk)
    x_jm = np.tile(np.arange(16, dtype=np.float32), chunk)
    cstE = np.concatenate([z_jm, y_jm, x_jm]).reshape(1, 40 * chunk)
    cstE = cstE.astype(np.float32)
    in_maps = []
    for core in range(NCORES):
        v, q = divmod(core, 4)
        r0, r1 = ROWS_PER_CORE * q, ROWS_PER_CORE * (q + 1)
        blk = lambda a: np.ascontiguousarray(a.reshape(P, ctot), np.float32)
        # G3[(zc*16+yc), (ch*16 + xc)] = grids[v, ch, zc, yc, xc]
        g3 = np.ascontiguousarray(
            grids[v].transpose(1, 2, 0, 3).reshape(P, 192), np.float32)
        in_maps.append({
            "xs": blk(coords[v, 0, r0:r1, :, 0]),
            "ys": blk(coords[v, 0, r0:r1, :, 1]),
            "rr": blk(rgb[v, 0, r0:r1, :, 0]),
            "gg": blk(rgb[v, 0, r0:r1, :, 1]),
            "bb": blk(rgb[v, 0, r0:r1, :, 2]),
            "g3": g3,
            "cstE": cstE,
        })
    return in_maps


def _run(grids, coords, rgb, trace=False):
    nc = _get_module()
    in_maps = _make_core_inputs(grids, coords, rgb)
    res = run_bass_kernel_spmd(nc, in_maps, core_ids=list(range(NCORES)),
                               trace=trace)
    outs = []
    for core in range(NCORES):
        o = res.results[core]["out"]
        outs.append(o.reshape(P, CTOT, 3).reshape(ROWS_PER_CORE, IMG_W, 3))
    full = np.empty((NVIEW, 1, IMG_H, IMG_W, 3), np.float32)
    for core in range(NCORES):
        v, q = divmod(core, 4)
        full[v, 0, ROWS_PER_CORE * q:ROWS_PER_CORE * (q + 1)] = outs[core]
    return full, res


def kernel(grids, coords, rgb):
    full, _ = _run(np.asarray(grids), np.asarray(coords), np.asarray(rgb))
    return full


# revision 20
# speedup vs baseline: 1.3221x; 1.1325x over previous
"""Bilateral-grid slice kernel for Trainium2 (8 NeuronCores, SPMD data-parallel).

Strategy (per core):
  - shard: view v = core//4 owns grids[v]; quarter q = core%4 owns image rows
    [270q, 270(q+1)) of the 1080-row image -> 518400 pixels per core.
  - pixels live in "block layout" [128 partitions, 4050 free] (pixel = p*4050+j).
  - trilinear interp of the (8,16,16) grid:
      hat weights  hz[8], hy[16], hx[16]  with hat(t) = relu(1-|t|)
      S = hz (x) hy  joint one-hot over the 128 (z,y)-cells  (DVE mul, bf16)
      S^T per 128-pixel tile via regular matmul against identity (cheaper
      than transpose-mode and keeps the PE HAM-warm), PSUM -> SBUF bf16
      V[px, (ch,x)] = S @ G3 on the PE in bf16  (G3 = grid [128, 192])
      V -> SBUF bf16 (ACT copy), w2 = V * hx (DVE bf16 2x), reduce over x
      out = A[:, i*4+j] affine-applied to rgb (GPSIMD tensor ops)
"""

import numpy as np
from contextlib import ExitStack

import concourse.bacc as bacc
import concourse.bass as bass
import concourse.tile as tile
import concourse.mybir as mybir
from concourse import masks
from concourse.bass_utils import run_bass_kernel_spmd

F32 = mybir.dt.float32
BF16 = mybir.dt.bfloat16
ALU = mybir.AluOpType
ACTFN = mybir.ActivationFunctionType

# problem geometry (hardcoded per contest rules)
NVIEW, L, GH, GW = 2, 8, 16, 16
IMG_H, IMG_W = 1080, 1920
NCORES = 8
P = 128

ROWS_PER_CORE = IMG_H // 4                      # 270
PIX_PER_CORE = ROWS_PER_CORE * IMG_W            # 518400
CTOT = PIX_PER_CORE // P                        # 4050
CHUNK = 162                                     # free-cols per chunk
NCHUNK = CTOT // CHUNK                          # 25
JGRP = 6                                        # j's per S-product group
SUBJ = 3                                        # j's per PSUM sub-tile
MACJ = 18                                       # j's per mul/reduce macro tile
NGRP = CHUNK // JGRP                            # 27

GRAY_R, GRAY_G, GRAY_B = 0.299, 0.587, 0.114


def _ap(base: bass.AP, offset_add: int, free_dims):
    """Raw AP on the same tensor/partitions as `base` with custom free dims."""
    return bass.AP(base.tensor, base.offset + offset_add, [base.ap[0]] + free_dims)


def build_module(ctot=CTOT, chunk=CHUNK, jgrp=JGRP):
    nchunk = ctot // chunk
    ngrp = chunk // jgrp
    assert ctot % chunk == 0 and chunk % jgrp == 0

    nc = bacc.Bacc("TRN2", target_bir_lowering=False, debug=False,
                   num_devices=NCORES)

    xs = nc.dram_tensor("xs", [P, ctot], F32, kind="ExternalInput").ap()
    ys = nc.dram_tensor("ys", [P, ctot], F32, kind="ExternalInput").ap()
    rr = nc.dram_tensor("rr", [P, ctot], F32, kind="ExternalInput").ap()
    gg = nc.dram_tensor("gg", [P, ctot], F32, kind="ExternalInput").ap()
    bb = nc.dram_tensor("bb", [P, ctot], F32, kind="ExternalInput").ap()
    g3d = nc.dram_tensor("g3", [P, 192], F32, kind="ExternalInput").ap()
    # hat-lattice constants pre-expanded per pixel-column position:
    # cstE[0, j*40 + 0:8]   = z lattice 0..7
    # cstE[0, j*40 + 8:24]  = y lattice 0..15
    # cstE[0, j*40 + 24:40] = x lattice 0..15
    cstE = nc.dram_tensor("cstE", [1, 40 * chunk], F32,
                          kind="ExternalInput").ap()
    out = nc.dram_tensor("out", [P, 3 * ctot], F32, kind="ExternalOutput").ap()

    with tile.TileContext(nc) as tc:
        with ExitStack() as ctx:
            cpool = ctx.enter_context(tc.tile_pool(name="const", bufs=1))
            inp = ctx.enter_context(tc.tile_pool(name="inp", bufs=2))
            hatp = ctx.enter_context(tc.tile_pool(name="hat", bufs=2))
            spool = ctx.enter_context(tc.tile_pool(name="sprod", bufs=3))
            stp = ctx.enter_context(
                tc.tile_pool(name="st_ps", bufs=2, space="PSUM"))
            stsb = ctx.enter_context(tc.tile_pool(name="st_sb", bufs=3))
            vps = ctx.enter_context(
                tc.tile_pool(name="v_ps", bufs=2, space="PSUM"))
            vsbp = ctx.enter_context(tc.tile_pool(name="v_sb", bufs=3))
            w2p = ctx.enter_context(tc.tile_pool(name="w2", bufs=2))
            apool = ctx.enter_context(tc.tile_pool(name="acc", bufs=2))
            opool = ctx.enter_context(tc.tile_pool(name="outb", bufs=2))

            # constants
            g3_f = cpool.tile([P, 192], F32)
            nc.sync.dma_start(g3_f[:], g3d)
            g3_sb = cpool.tile([P, 192], BF16)
            nc.vector.tensor_copy(g3_sb[:], g3_f[:])
            # expanded lattice constants, bf16, one per (j, cell) so the hat
            # STT reads step-1 (keeps the DVE 2x mode eligible)
            cstE_f = cpool.tile([P, 40 * chunk], F32)
            nc.sync.dma_start(cstE_f[:],
                              cstE[0:1, :].to_broadcast((P, 40 * chunk)))
            cstE_sb = cpool.tile([P, 40 * chunk], BF16)
            nc.vector.tensor_copy(cstE_sb[:], cstE_f[:])
            ident = cpool.tile([P, P], F32)
            masks.make_identity(nc, ident[:])
            ident_b = cpool.tile([P, P], BF16)
            nc.vector.tensor_copy(ident_b[:], ident[:])

            for ci in range(nchunk):
                cb = ci * chunk
                xt = inp.tile([P, chunk], F32, tag="xt")
                nc.sync.dma_start(xt[:], xs[:, cb:cb + chunk])
                yt = inp.tile([P, chunk], F32, tag="yt")
                nc.sync.dma_start(yt[:], ys[:, cb:cb + chunk])
                rt = inp.tile([P, chunk], F32, tag="rt")
                nc.sync.dma_start(rt[:], rr[:, cb:cb + chunk])
                gt = inp.tile([P, chunk], F32, tag="gt")
                nc.sync.dma_start(gt[:], gg[:, cb:cb + chunk])
                bt = inp.tile([P, chunk], F32, tag="bt")
                nc.sync.dma_start(bt[:], bb[:, cb:cb + chunk])

                # gray precursor (z = t2 * 0.587*7 folded into the hat STT)
                t1 = inp.tile([P, chunk], F32, tag="t1")
                nc.vector.scalar_tensor_tensor(
                    t1[:], rt[:], GRAY_R / GRAY_G, gt[:],
                    op0=ALU.mult, op1=ALU.add)
                t2 = inp.tile([P, chunk], F32, tag="t2")
                nc.vector.scalar_tensor_tensor(
                    t2[:], bt[:], GRAY_B / GRAY_G, t1[:],
                    op0=ALU.mult, op1=ALU.add)
                t2b = inp.tile([P, chunk], BF16, tag="t2b")
                nc.vector.tensor_copy(t2b[:], t2[:])
                xtb = inp.tile([P, chunk], BF16, tag="xtb")
                nc.vector.tensor_copy(xtb[:], xt[:])
                ytb = inp.tile([P, chunk], BF16, tag="ytb")
                nc.vector.tensor_copy(ytb[:], yt[:])

                # hat argument tiles, free layout (j, cell) j-major, bf16
                hz = hatp.tile([P, 8 * chunk], BF16, tag="hz")
                nc.vector.scalar_tensor_tensor(
                    hz[:].rearrange("p (j z) -> p j z", z=8),
                    t2b[:].unsqueeze(2).broadcast_to((P, chunk, 8)),
                    GRAY_G * (L - 1),
                    cstE_sb[:, 0:8 * chunk].rearrange(
                        "p (j z) -> p j z", z=8),
                    op0=ALU.mult, op1=ALU.subtract)
                hy = hatp.tile([P, 16 * chunk], BF16, tag="hy")
                nc.vector.scalar_tensor_tensor(
                    hy[:].rearrange("p (j y) -> p j y", y=16),
                    ytb[:].unsqueeze(2).broadcast_to((P, chunk, 16)),
                    float(GH - 1),
                    cstE_sb[:, 8 * chunk:24 * chunk].rearrange(
                        "p (j y) -> p j y", y=16),
                    op0=ALU.mult, op1=ALU.subtract)
                hx = hatp.tile([P, 16 * chunk], BF16, tag="hx")
                nc.vector.scalar_tensor_tensor(
                    hx[:].rearrange("p (j x) -> p j x", x=16),
                    xtb[:].unsqueeze(2).broadcast_to((P, chunk, 16)),
                    float(GW - 1),
                    cstE_sb[:, 24 * chunk:40 * chunk].rearrange(
                        "p (j x) -> p j x", x=16),
                    op0=ALU.mult, op1=ALU.subtract)

                # hat(t) = relu(1 - |t|), in place on ACT
                for h in (hz, hy, hx):
                    nc.scalar.activation(h[:], h[:], ACTFN.Abs)
                    nc.scalar.activation(h[:], h[:], ACTFN.Relu,
                                         bias=1.0, scale=-1.0)

                a_ch = apool.tile([P, 12 * chunk], F32, tag="a_ch")

                for mg in range(chunk // MACJ):
                    mb = mg * MACJ
                    # V for MACJ j's accumulates here (bf16), then one big
                    # mul + reduce amortizes the per-instruction overhead
                    v_sb = vsbp.tile([P, MACJ * 192], BF16)

                    for g in range(MACJ // jgrp):
                        jb = mb + g * jgrp
                        # S one-hot product [pix, (j, z, y)] bf16
                        sg = spool.tile([P, jgrp * P], BF16, tag="sg")
                        nc.vector.tensor_tensor(
                            sg[:].rearrange("p (j z y) -> p j z y",
                                            j=jgrp, z=8),
                            _ap(hz[:], jb * 8, [[8, jgrp], [1, 8], [0, 16]]),
                            _ap(hy[:], jb * 16,
                                [[16, jgrp], [0, 8], [1, 16]]),
                            op=ALU.mult)

                        # V PSUM tile for the whole 6j group (3 banks);
                        # 256-elem slots keep each 192-wide matmul output
                        # inside one 512-float PSUM bank
                        vt = vps.tile([P, jgrp * 256], F32)
                        for h in range(jgrp // SUBJ):
                            qb = h * SUBJ
                            # S^T via matmuls against identity (PSUM out)
                            st_ps = stp.tile([P, SUBJ * P], F32)
                            for q in range(SUBJ):
                                nc.tensor.matmul(
                                    _ap(st_ps[:], q * P, [[1, P]]),
                                    lhsT=sg[:, (qb + q) * P:(qb + q + 1) * P],
                                    rhs=ident_b[:], start=True, stop=True)
                            st_sb = stsb.tile([P, SUBJ * P], BF16)
                            nc.scalar.copy(st_sb[:], st_ps[:])

                            # V[pix, (ch, x)] = S @ G3, bf16 weights (FWL)
                            for q in range(SUBJ):
                                nc.tensor.matmul(
                                    _ap(vt[:], (qb + q) * 256, [[1, 192]]),
                                    lhsT=_ap(st_sb[:], q * P, [[1, P]]),
                                    rhs=g3_sb[:], start=True, stop=True)
                        nc.scalar.copy(
                            _ap(v_sb[:], g * jgrp * 192,
                                [[192, jgrp], [1, 192]]),
                            _ap(vt[:], 0, [[256, jgrp], [1, 192]]))

                    # w2 = V * hx (bf16 2x; inner dim = 16 contiguous x)
                    w2 = w2p.tile([P, MACJ * 192], BF16)
                    nc.vector.tensor_tensor(
                        _ap(w2[:], 0, [[192, MACJ], [16, 12], [1, 16]]),
                        _ap(v_sb[:], 0, [[192, MACJ], [16, 12], [1, 16]]),
                        _ap(hx[:], mb * 16, [[16, MACJ], [0, 12], [1, 16]]),
                        op=ALU.mult)
                    # x-reduce as a TT-add tree: levels 1-3 run in the DVE
                    # bf16 2x mode (both operands half-size, step-1 inner),
                    # beating tensor_reduce's fixed 1x; final level -> fp32
                    t8 = w2p.tile([P, MACJ * 96], BF16, tag="t8")
                    nc.vector.tensor_tensor(
                        _ap(t8[:], 0, [[96, MACJ], [8, 12], [1, 8]]),
                        _ap(w2[:], 0, [[192, MACJ], [16, 12], [1, 8]]),
                        _ap(w2[:], 8, [[192, MACJ], [16, 12], [1, 8]]),
                        op=ALU.add)
                    t4 = w2p.tile([P, MACJ * 48], BF16, tag="t4")
                    nc.vector.tensor_tensor(
                        _ap(t4[:], 0, [[48, MACJ], [4, 12], [1, 4]]),
                        _ap(t8[:], 0, [[96, MACJ], [8, 12], [1, 4]]),
                        _ap(t8[:], 4, [[96, MACJ], [8, 12], [1, 4]]),
                        op=ALU.add)
                    t2r = w2p.tile([P, MACJ * 24], BF16, tag="t2r")
                    nc.vector.tensor_tensor(
                        _ap(t2r[:], 0, [[24, MACJ], [2, 12], [1, 2]]),
                        _ap(t4[:], 0, [[48, MACJ], [4, 12], [1, 2]]),
                        _ap(t4[:], 2, [[48, MACJ], [4, 12], [1, 2]]),
                        op=ALU.add)
                    nc.vector.tensor_tensor(
                        _ap(a_ch[:], mb * 12, [[12, MACJ], [1, 12]]),
                        _ap(t2r[:], 0, [[24, MACJ], [2, 12]]),
                        _ap(t2r[:], 1, [[24, MACJ], [2, 12]]),
                        op=ALU.add)

                # affine apply on GPSIMD:
                # out_i = A[4i]*r + A[4i+1]*g + A[4i+2]*b + A[4i+3]
                ot = opool.tile([P, 3 * chunk], F32, tag="ot")
                rgbt = (rt, gt, bt)
                for i in range(3):
                    m = []
                    for j in range(3):
                        mj = opool.tile([P, chunk], F32, tag=f"m{j}")
                        nc.gpsimd.tensor_tensor(
                            mj[:],
                            _ap(a_ch[:], 4 * i + j, [[12, chunk]]),
                            rgbt[j][:], op=ALU.mult)
                        m.append(mj)
                    s1 = opool.tile([P, chunk], F32, tag="s1")
                    nc.gpsimd.tensor_tensor(s1[:], m[0][:], m[1][:], op=ALU.add)
                    s2 = opool.tile([P, chunk], F32, tag="s2")
                    nc.gpsimd.tensor_tensor(
                        s2[:], m[2][:],
                        _ap(a_ch[:], 4 * i + 3, [[12, chunk]]), op=ALU.add)
                    nc.gpsimd.tensor_tensor(
                        _ap(ot[:], i, [[3, chunk]]), s1[:], s2[:], op=ALU.add)

                nc.sync.dma_start(out[:, 3 * cb:3 * (cb + chunk)], ot[:])

    nc.compile()
    return nc


_NC_CACHE = {}


def _get_module():
    key = (CTOT, CHUNK, JGRP)
    if key not in _NC_CACHE:
        _NC_CACHE[key] = build_module()
    return _NC_CACHE[key]


def _make_core_inputs(grids, coords, rgb, ctot=CTOT, chunk=CHUNK):
    """Per-core input dicts (numpy layout prep only)."""
    # j-major lattices, one copy per pixel column (keeps STT reads step-1)
    z_jm = np.tile(np.arange(8, dtype=np.float32), chunk)
    y_jm = np.tile(np.arange(16, dtype=np.float32), chunk)
    x_jm = np.tile(np.arange(16, dtype=np.float32), chunk)
    cstE = np.concatenate([z_jm, y_jm, x_jm]).reshape(1, 40 * chunk)
    cstE = cstE.astype(np.float32)
    in_maps = []
    for core in range(NCORES):
        v, q = divmod(core, 4)
        r0, r1 = ROWS_PER_CORE * q, ROWS_PER_CORE * (q + 1)
        blk = lambda a: np.ascontiguousarray(a.reshape(P, ctot), np.float32)
        # G3[(zc*16+yc), (ch*16 + xc)] = grids[v, ch, zc, yc, xc]
        g3 = np.ascontiguousarray(
            grids[v].transpose(1, 2, 0, 3).reshape(P, 192), np.float32)
        in_maps.append({
            "xs": blk(coords[v, 0, r0:r1, :, 0]),
            "ys": blk(coords[v, 0, r0:r1, :, 1]),
            "rr": blk(rgb[v, 0, r0:r1, :, 0]),
            "gg": blk(rgb[v, 0, r0:r1, :, 1]),
            "bb": blk(rgb[v, 0, r0:r1, :, 2]),
            "g3": g3,
            "cstE": cstE,
        })
    return in_maps


def _run(grids, coords, rgb, trace=False):
    nc = _get_module()
    in_maps = _make_core_inputs(grids, coords, rgb)
    res = run_bass_kernel_spmd(nc, in_maps, core_ids=list(range(NCORES)),
                               trace=trace)
    outs = []
    for core in range(NCORES):
        o = res.results[core]["out"]
        outs.append(o.reshape(P, CTOT, 3).reshape(ROWS_PER_CORE, IMG_W, 3))
    full = np.empty((NVIEW, 1, IMG_H, IMG_W, 3), np.float32)
    for core in range(NCORES):
        v, q = divmod(core, 4)
        full[v, 0, ROWS_PER_CORE * q:ROWS_PER_CORE * (q + 1)] = outs[core]
    return full, res


def kernel(grids, coords, rgb):
    full, _ = _run(np.asarray(grids), np.asarray(coords), np.asarray(rgb))
    return full


# revision 24
# speedup vs baseline: 1.3779x; 1.0422x over previous
"""Bilateral-grid slice kernel for Trainium2 (8 NeuronCores, SPMD data-parallel).

Strategy (per core):
  - shard: view v = core//4 owns grids[v]; quarter q = core%4 owns image rows
    [270q, 270(q+1)) of the 1080-row image -> 518400 pixels per core.
  - pixels live in "block layout" [128 partitions, 4050 free] (pixel = p*4050+j).
  - trilinear interp of the (8,16,16) grid:
      hat weights  hz[8], hy[16], hx[16]  with hat(t) = relu(1-|t|)
      S = hz (x) hy  joint one-hot over the 128 (z,y)-cells  (DVE mul, bf16)
      S^T per 128-pixel tile via regular matmul against identity (cheaper
      than transpose-mode and keeps the PE HAM-warm), PSUM -> SBUF bf16
      V[px, (ch,x)] = S @ G3 on the PE in bf16  (G3 = grid [128, 192])
      V -> SBUF bf16 (ACT copy), w2 = V * hx (DVE bf16 2x), reduce over x
      out = A[:, i*4+j] affine-applied to rgb (GPSIMD tensor ops)
"""

import numpy as np
from contextlib import ExitStack

import concourse.bacc as bacc
import concourse.bass as bass
import concourse.tile as tile
import concourse.mybir as mybir
from concourse import masks
from concourse.bass_utils import run_bass_kernel_spmd

F32 = mybir.dt.float32
BF16 = mybir.dt.bfloat16
ALU = mybir.AluOpType
ACTFN = mybir.ActivationFunctionType

# problem geometry (hardcoded per contest rules)
NVIEW, L, GH, GW = 2, 8, 16, 16
IMG_H, IMG_W = 1080, 1920
NCORES = 8
P = 128

ROWS_PER_CORE = IMG_H // 4                      # 270
PIX_PER_CORE = ROWS_PER_CORE * IMG_W            # 518400
CTOT = PIX_PER_CORE // P                        # 4050
CHUNK = 162                                     # free-cols per chunk
NCHUNK = CTOT // CHUNK                          # 25
JGRP = 6                                        # j's per S-product group
SUBJ = 3                                        # j's per PSUM sub-tile
MACJ = 18                                       # j's per mul/reduce macro tile
NGRP = CHUNK // JGRP                            # 27

GRAY_R, GRAY_G, GRAY_B = 0.299, 0.587, 0.114


def _ap(base: bass.AP, offset_add: int, free_dims):
    """Raw AP on the same tensor/partitions as `base` with custom free dims."""
    return bass.AP(base.tensor, base.offset + offset_add, [base.ap[0]] + free_dims)


def build_module(ctot=CTOT, chunk=CHUNK, jgrp=JGRP):
    nchunk = ctot // chunk
    ngrp = chunk // jgrp
    assert ctot % chunk == 0 and chunk % jgrp == 0

    nc = bacc.Bacc("TRN2", target_bir_lowering=False, debug=False,
                   num_devices=NCORES)

    xs = nc.dram_tensor("xs", [P, ctot], F32, kind="ExternalInput").ap()
    ys = nc.dram_tensor("ys", [P, ctot], F32, kind="ExternalInput").ap()
    rr = nc.dram_tensor("rr", [P, ctot], F32, kind="ExternalInput").ap()
    gg = nc.dram_tensor("gg", [P, ctot], F32, kind="ExternalInput").ap()
    bb = nc.dram_tensor("bb", [P, ctot], F32, kind="ExternalInput").ap()
    g3d = nc.dram_tensor("g3", [P, 192], F32, kind="ExternalInput").ap()
    # hat-lattice constants pre-expanded per pixel-column position:
    # cstE[0, j*40 + 0:8]   = z lattice 0..7
    # cstE[0, j*40 + 8:24]  = y lattice 0..15
    # cstE[0, j*40 + 24:40] = x lattice 0..15
    cstE = nc.dram_tensor("cstE", [1, 40 * chunk], F32,
                          kind="ExternalInput").ap()
    out = nc.dram_tensor("out", [P, 3 * ctot], F32, kind="ExternalOutput").ap()

    with tile.TileContext(nc) as tc:
        with ExitStack() as ctx:
            cpool = ctx.enter_context(tc.tile_pool(name="const", bufs=1))
            inp = ctx.enter_context(tc.tile_pool(name="inp", bufs=2))
            hatp = ctx.enter_context(tc.tile_pool(name="hat", bufs=2))
            spool = ctx.enter_context(tc.tile_pool(name="sprod", bufs=3))
            stp = ctx.enter_context(
                tc.tile_pool(name="st_ps", bufs=2, space="PSUM"))
            stsb = ctx.enter_context(tc.tile_pool(name="st_sb", bufs=3))
            vps = ctx.enter_context(
                tc.tile_pool(name="v_ps", bufs=1, space="PSUM"))
            vsbp = ctx.enter_context(tc.tile_pool(name="v_sb", bufs=3))
            w2p = ctx.enter_context(tc.tile_pool(name="w2", bufs=2))
            apool = ctx.enter_context(tc.tile_pool(name="acc", bufs=2))
            opool = ctx.enter_context(tc.tile_pool(name="outb", bufs=2))

            # constants
            g3_f = cpool.tile([P, 192], F32)
            nc.sync.dma_start(g3_f[:], g3d)
            g3_sb = cpool.tile([P, 192], BF16)
            nc.vector.tensor_copy(g3_sb[:], g3_f[:])
            # expanded lattice constants, bf16, one per (j, cell) so the hat
            # STT reads step-1 (keeps the DVE 2x mode eligible)
            cstE_f = cpool.tile([P, 40 * chunk], F32)
            nc.sync.dma_start(cstE_f[:],
                              cstE[0:1, :].to_broadcast((P, 40 * chunk)))
            cstE_sb = cpool.tile([P, 40 * chunk], BF16)
            nc.vector.tensor_copy(cstE_sb[:], cstE_f[:])
            ident = cpool.tile([P, P], F32)
            masks.make_identity(nc, ident[:])
            ident_b = cpool.tile([P, P], BF16)
            nc.vector.tensor_copy(ident_b[:], ident[:])

            for ci in range(nchunk):
                cb = ci * chunk
                xt = inp.tile([P, chunk], F32, tag="xt")
                nc.sync.dma_start(xt[:], xs[:, cb:cb + chunk])
                yt = inp.tile([P, chunk], F32, tag="yt")
                nc.sync.dma_start(yt[:], ys[:, cb:cb + chunk])
                rt = inp.tile([P, chunk], F32, tag="rt")
                nc.sync.dma_start(rt[:], rr[:, cb:cb + chunk])
                gt = inp.tile([P, chunk], F32, tag="gt")
                nc.sync.dma_start(gt[:], gg[:, cb:cb + chunk])
                bt = inp.tile([P, chunk], F32, tag="bt")
                nc.sync.dma_start(bt[:], bb[:, cb:cb + chunk])

                # gray precursor (z = t2 * 0.587*7 folded into the hat STT)
                t1 = inp.tile([P, chunk], F32, tag="t1")
                nc.vector.scalar_tensor_tensor(
                    t1[:], rt[:], GRAY_R / GRAY_G, gt[:],
                    op0=ALU.mult, op1=ALU.add)
                t2 = inp.tile([P, chunk], F32, tag="t2")
                nc.vector.scalar_tensor_tensor(
                    t2[:], bt[:], GRAY_B / GRAY_G, t1[:],
                    op0=ALU.mult, op1=ALU.add)
                t2b = inp.tile([P, chunk], BF16, tag="t2b")
                nc.vector.tensor_copy(t2b[:], t2[:])
                xtb = inp.tile([P, chunk], BF16, tag="xtb")
                nc.vector.tensor_copy(xtb[:], xt[:])
                ytb = inp.tile([P, chunk], BF16, tag="ytb")
                nc.vector.tensor_copy(ytb[:], yt[:])

                # hat argument tiles, free layout (j, cell) j-major, bf16
                hz = hatp.tile([P, 8 * chunk], BF16, tag="hz")
                nc.vector.scalar_tensor_tensor(
                    hz[:].rearrange("p (j z) -> p j z", z=8),
                    t2b[:].unsqueeze(2).broadcast_to((P, chunk, 8)),
                    GRAY_G * (L - 1),
                    cstE_sb[:, 0:8 * chunk].rearrange(
                        "p (j z) -> p j z", z=8),
                    op0=ALU.mult, op1=ALU.subtract)
                hy = hatp.tile([P, 16 * chunk], BF16, tag="hy")
                nc.vector.scalar_tensor_tensor(
                    hy[:].rearrange("p (j y) -> p j y", y=16),
                    ytb[:].unsqueeze(2).broadcast_to((P, chunk, 16)),
                    float(GH - 1),
                    cstE_sb[:, 8 * chunk:24 * chunk].rearrange(
                        "p (j y) -> p j y", y=16),
                    op0=ALU.mult, op1=ALU.subtract)
                hx = hatp.tile([P, 16 * chunk], BF16, tag="hx")
                nc.vector.scalar_tensor_tensor(
                    hx[:].rearrange("p (j x) -> p j x", x=16),
                    xtb[:].unsqueeze(2).broadcast_to((P, chunk, 16)),
                    float(GW - 1),
                    cstE_sb[:, 24 * chunk:40 * chunk].rearrange(
                        "p (j x) -> p j x", x=16),
                    op0=ALU.mult, op1=ALU.subtract)

                # hat(t) = relu(1 - |t|), in place on ACT
                for h in (hz, hy, hx):
                    nc.scalar.activation(h[:], h[:], ACTFN.Abs)
                    nc.scalar.activation(h[:], h[:], ACTFN.Relu,
                                         bias=1.0, scale=-1.0)

                a_ch = apool.tile([P, 12 * chunk], F32, tag="a_ch")

                for mg in range(chunk // MACJ):
                    mb = mg * MACJ
                    # V for MACJ j's accumulates here (bf16), then one big
                    # mul + reduce amortizes the per-instruction overhead
                    v_sb = vsbp.tile([P, MACJ * 192], BF16)

                    for g in range(MACJ // jgrp):
                        jb = mb + g * jgrp
                        # S one-hot product [pix, (j, z, y)] bf16
                        sg = spool.tile([P, jgrp * P], BF16, tag="sg")
                        nc.vector.tensor_tensor(
                            sg[:].rearrange("p (j z y) -> p j z y",
                                            j=jgrp, z=8),
                            _ap(hz[:], jb * 8, [[8, jgrp], [1, 8], [0, 16]]),
                            _ap(hy[:], jb * 16,
                                [[16, jgrp], [0, 8], [1, 16]]),
                            op=ALU.mult)

                        # S^T via matmuls against identity, full 6j PSUM
                        # tile (2 banks), one copy
                        st_ps = stp.tile([P, jgrp * P], F32)
                        for q in range(jgrp):
                            nc.tensor.matmul(
                                _ap(st_ps[:], q * P, [[1, P]]),
                                lhsT=sg[:, q * P:(q + 1) * P],
                                rhs=ident_b[:], start=True, stop=True)
                        st_sb = stsb.tile([P, jgrp * P], BF16)
                        nc.scalar.copy(st_sb[:], st_ps[:])

                        # V[pix, (ch, x)] = S @ G3, bf16 weights (FWL).
                        # 256-elem slots keep each 192-wide matmul output
                        # inside one 512-float PSUM bank; 3 banks, 1 buf
                        vt = vps.tile([P, jgrp * 256], F32)
                        for q in range(jgrp):
                            nc.tensor.matmul(
                                _ap(vt[:], q * 256, [[1, 192]]),
                                lhsT=_ap(st_sb[:], q * P, [[1, P]]),
                                rhs=g3_sb[:], start=True, stop=True)
                        nc.scalar.copy(
                            _ap(v_sb[:], g * jgrp * 192,
                                [[192, jgrp], [1, 192]]),
                            _ap(vt[:], 0, [[256, jgrp], [1, 192]]))

                    # w2 = V * hx (bf16 2x; inner dim = 16 contiguous x)
                    w2 = w2p.tile([P, MACJ * 192], BF16)
                    nc.vector.tensor_tensor(
                        _ap(w2[:], 0, [[192, MACJ], [16, 12], [1, 16]]),
                        _ap(v_sb[:], 0, [[192, MACJ], [16, 12], [1, 16]]),
                        _ap(hx[:], mb * 16, [[16, MACJ], [0, 12], [1, 16]]),
                        op=ALU.mult)
                    # x-reduce as a TT-add tree: levels 1-3 run in the DVE
                    # bf16 2x mode (both operands half-size, step-1 inner),
                    # beating tensor_reduce's fixed 1x; final level -> fp32
                    t8 = w2p.tile([P, MACJ * 96], BF16, tag="t8")
                    nc.vector.tensor_tensor(
                        _ap(t8[:], 0, [[96, MACJ], [8, 12], [1, 8]]),
                        _ap(w2[:], 0, [[192, MACJ], [16, 12], [1, 8]]),
                        _ap(w2[:], 8, [[192, MACJ], [16, 12], [1, 8]]),
                        op=ALU.add)
                    t4 = w2p.tile([P, MACJ * 48], BF16, tag="t4")
                    nc.vector.tensor_tensor(
                        _ap(t4[:], 0, [[48, MACJ], [4, 12], [1, 4]]),
                        _ap(t8[:], 0, [[96, MACJ], [8, 12], [1, 4]]),
                        _ap(t8[:], 4, [[96, MACJ], [8, 12], [1, 4]]),
                        op=ALU.add)
                    t2r = w2p.tile([P, MACJ * 24], BF16, tag="t2r")
                    nc.vector.tensor_tensor(
                        _ap(t2r[:], 0, [[24, MACJ], [2, 12], [1, 2]]),
                        _ap(t4[:], 0, [[48, MACJ], [4, 12], [1, 2]]),
                        _ap(t4[:], 2, [[48, MACJ], [4, 12], [1, 2]]),
                        op=ALU.add)
                    nc.vector.tensor_tensor(
                        _ap(a_ch[:], mb * 12, [[12, MACJ], [1, 12]]),
                        _ap(t2r[:], 0, [[24, MACJ], [2, 12]]),
                        _ap(t2r[:], 1, [[24, MACJ], [2, 12]]),
                        op=ALU.add)

                # affine apply on GPSIMD:
                # out_i = A[4i]*r + A[4i+1]*g + A[4i+2]*b + A[4i+3]
                ot = opool.tile([P, 3 * chunk], F32, tag="ot")
                rgbt = (rt, gt, bt)
                for i in range(3):
                    m = []
                    for j in range(3):
                        mj = opool.tile([P, chunk], F32, tag=f"m{j}")
                        nc.gpsimd.tensor_tensor(
                            mj[:],
                            _ap(a_ch[:], 4 * i + j, [[12, chunk]]),
                            rgbt[j][:], op=ALU.mult)
                        m.append(mj)
                    s1 = opool.tile([P, chunk], F32, tag="s1")
                    nc.gpsimd.tensor_tensor(s1[:], m[0][:], m[1][:], op=ALU.add)
                    s2 = opool.tile([P, chunk], F32, tag="s2")
                    nc.gpsimd.tensor_tensor(
                        s2[:], m[2][:],
                        _ap(a_ch[:], 4 * i + 3, [[12, chunk]]), op=ALU.add)
                    nc.gpsimd.tensor_tensor(
                        _ap(ot[:], i, [[3, chunk]]), s1[:], s2[:], op=ALU.add)

                nc.sync.dma_start(out[:, 3 * cb:3 * (cb + chunk)], ot[:])

    nc.compile()
    return nc


_NC_CACHE = {}


def _get_module():
    key = (CTOT, CHUNK, JGRP)
    if key not in _NC_CACHE:
        _NC_CACHE[key] = build_module()
    return _NC_CACHE[key]


def _make_core_inputs(grids, coords, rgb, ctot=CTOT, chunk=CHUNK):
    """Per-core input dicts (numpy layout prep only)."""
    # j-major lattices, one copy per pixel column (keeps STT reads step-1)
    z_jm = np.tile(np.arange(8, dtype=np.float32), chunk)
    y_jm = np.tile(np.arange(16, dtype=np.float32), chunk)
    x_jm = np.tile(np.arange(16, dtype=np.float32), chunk)
    cstE = np.concatenate([z_jm, y_jm, x_jm]).reshape(1, 40 * chunk)
    cstE = cstE.astype(np.float32)
    in_maps = []
    for core in range(NCORES):
        v, q = divmod(core, 4)
        r0, r1 = ROWS_PER_CORE * q, ROWS_PER_CORE * (q + 1)
        blk = lambda a: np.ascontiguousarray(a.reshape(P, ctot), np.float32)
        # G3[(zc*16+yc), (ch*16 + xc)] = grids[v, ch, zc, yc, xc]
        g3 = np.ascontiguousarray(
            grids[v].transpose(1, 2, 0, 3).reshape(P, 192), np.float32)
        in_maps.append({
            "xs": blk(coords[v, 0, r0:r1, :, 0]),
            "ys": blk(coords[v, 0, r0:r1, :, 1]),
            "rr": blk(rgb[v, 0, r0:r1, :, 0]),
            "gg": blk(rgb[v, 0, r0:r1, :, 1]),
            "bb": blk(rgb[v, 0, r0:r1, :, 2]),
            "g3": g3,
            "cstE": cstE,
        })
    return in_maps


def _run(grids, coords, rgb, trace=False):
    nc = _get_module()
    in_maps = _make_core_inputs(grids, coords, rgb)
    res = run_bass_kernel_spmd(nc, in_maps, core_ids=list(range(NCORES)),
                               trace=trace)
    outs = []
    for core in range(NCORES):
        o = res.results[core]["out"]
        outs.append(o.reshape(P, CTOT, 3).reshape(ROWS_PER_CORE, IMG_W, 3))
    full = np.empty((NVIEW, 1, IMG_H, IMG_W, 3), np.float32)
    for core in range(NCORES):
        v, q = divmod(core, 4)
        full[v, 0, ROWS_PER_CORE * q:ROWS_PER_CORE * (q + 1)] = outs[core]
    return full, res


def kernel(grids, coords, rgb):
    full, _ = _run(np.asarray(grids), np.asarray(coords), np.asarray(rgb))
    return full
